# revision 19
# baseline (speedup 1.0000x reference)
"""GAT attention layer (EEGGraphAttentionLayer) for Trainium2, 8 NeuronCores.

reference math:
    Wh = h @ w                         # (8192, 64)
    e  = leaky_relu((Wh@a_src) + (Wh@a_dst).T, slope=0.2)   # (8192, 8192)
    att = where(adj > 0, e, -1e12)
    out = softmax(att, axis=1)

Sharding: rows of adj/out across 8 cores (1024 rows each); row softmax is
core-local. Each core recomputes the column-score vector s2 = h @ (w@a_dst)
(an N-vector) from the full h instead of communicating. h and w ride as
fp16 and adj as bf16 (sign-exact dtype casts) to cut HBM traffic; the
output is produced in bf16 on device and upcast to f32 on host.

Measured TRN2 engine reality (NTFF traces): DVE tensor_scalar runs 4x at
16-bit / 2x at f32, tensor_tensor 2x at 16-bit, scalar_tensor_tensor is
always 1x; ACT runs 1x at any dtype. So the pipeline keeps everything
16-bit, avoids 3-operand DVE ops, and injects the adjacency mask as a
-1e4 additive penalty before the exp:

Per-core device pipeline (row tile = [128, 8192]):
    wa12 = wT.T @ [a_src|a_dst]                        (PE)
    s1c[:, t] = hsT_tile.T @ wa1  (f32)                (PE fp16 in)
    bc2[j] = s2[j]  fp16, bcast over partitions        (PE fp16, 16 chunks)
    mn  = (adj is_le 0) * -1e4    fp16                 (DVE ts 2-op, 4x)
    l   = leaky_relu(bc2 + s1c_t) fp16                 (ACT Prelu [0,FSP);
                                                        DVE ts-add, ts-mul,
                                                        tt-max for [FSP,N))
    l  += mn                                           (DVE tt-add, 2x)
    p   = Exp(l - MSHIFT) -> bf16, S = rowsum accum    (ACT)
    out = p * (1/S)               bf16                 (DVE ts 4x) then DMA

Key points:
  - Masking by additive -1e4 penalty: exp(l - 1e4 - 32) underflows to
    exactly 0.0, matching the reference's -1e12 masking, and the row sum
    rides the Exp accum_out for free.
  - Fixed softmax shift MSHIFT (softmax is shift-invariant; scores are
    bounded: |s1|,|s2| < 10) removes the row-max reduction entirely.
  - Scores in fp16: |x| <= ~20 so ulp <= 2^-6; end-to-end emulated
    fro_rel vs the f64 oracle is ~3e-3 (gate is 2e-2).
  - ACT runs only Prelu/Exp, which share one activation table (pre-warmed
    in setup); adj loads ride the SP HWDGE ring, stores the ACT ring.
"""
import os
import sys

for _p in (
    "/opt/trn_rl_repo",
    "/root/.axon_site/_ro/trn_rl_repo",
):
    if os.path.isdir(_p) and _p not in sys.path:
        sys.path.append(_p)

import numpy as np
import ml_dtypes


def _install_profile_shim():
    """bass_utils' trace path imports antenv.axon_hooks, which this image
    lacks. Provide it (with the ctypes hook into libaxon if available) so a
    BASS_TRACE=1 run profiles instead of crashing. No-op on any failure."""
    import contextlib
    import ctypes
    import types

    if "antenv.axon_hooks" in sys.modules:
        return
    try:
        import antenv
    except ImportError:
        return

    def _make_hook(so_path):
        try:
            lib = ctypes.CDLL(so_path)
        except OSError:
            return None
        if not hasattr(lib, "axon_start_nrt_profile"):
            return None
        lib.axon_start_nrt_profile.argtypes = [
            ctypes.POINTER(ctypes.c_int64),
            ctypes.c_size_t,
        ]
        lib.axon_start_nrt_profile.restype = ctypes.c_int64
        lib.axon_stop_nrt_profile.argtypes = [ctypes.c_char_p]
        lib.axon_stop_nrt_profile.restype = ctypes.c_int64

        @contextlib.contextmanager
        def _hook(output_dir, device_ids):
            import jax

            jax.devices()
            if device_ids:
                ids = (ctypes.c_int64 * len(device_ids))(*device_ids)
                rc = lib.axon_start_nrt_profile(ids, len(device_ids))
            else:
                rc = lib.axon_start_nrt_profile(None, 0)
            if rc != 0:
                raise RuntimeError(f"axon_start_nrt_profile rc={rc}")
            try:
                yield
            finally:
                n = lib.axon_stop_nrt_profile(str(output_dir).encode())
                print(f"profile: {n} file(s) -> {output_dir}", file=sys.stderr)

        return _hook

    hook = [_make_hook("/opt/axon/libaxon_pjrt.so")]
    mod = types.ModuleType("antenv.axon_hooks")
    mod.set_axon_ntff_profile_hook = lambda h: hook.__setitem__(0, h)
    mod.get_axon_ntff_profile_hook = lambda: hook[0]
    sys.modules["antenv.axon_hooks"] = mod
    antenv.axon_hooks = mod


try:
    _install_profile_shim()
except Exception:
    pass

import concourse.bacc as bacc
import concourse.tile as tile
import concourse.bass as bass
from concourse import mybir
from concourse.bass_utils import run_bass_kernel_spmd

N, F_IN, F_OUT = 8192, 128, 64
NCORES = 8
R = N // NCORES          # rows per core (1024)
P = 128                  # SBUF partitions
RT = R // P              # row tiles per core (8)
H = N // 2               # column half (4096)
MSHIFT = 32.0            # fixed softmax shift: scores are in ~[-19, 19]
ALPHA = 0.2              # leaky relu negative slope
FSP = 5440               # lrelu split: ACT Prelu [0, FSP), DVE [FSP, N)
F32 = mybir.dt.float32
F16 = mybir.dt.float16
BF16 = mybir.dt.bfloat16
MPEN = -10000.0          # additive mask penalty (exp underflows to 0)
ADJ_DT = mybir.dt.bfloat16
ADJ_NP = ml_dtypes.bfloat16
AF = mybir.ActivationFunctionType
ALU = mybir.AluOpType

_CACHED_NC = None
LAST_RESULT = None       # BassKernelResults of the most recent run (for tests)


def build_nc():
    nc = bacc.Bacc("TRN2", target_bir_lowering=False)
    hT_d = nc.dram_tensor("hT", [F_IN, N], F16, kind="ExternalInput")
    hsT_d = nc.dram_tensor("hsT", [F_IN, R], F16, kind="ExternalInput")
    adj_d = nc.dram_tensor("adj", [R, N], ADJ_DT, kind="ExternalInput")
    wT_d = nc.dram_tensor("wT", [F_OUT, F_IN], F32, kind="ExternalInput")
    a_d = nc.dram_tensor("a", [2 * F_OUT, 1], F32, kind="ExternalInput")
    out_d = nc.dram_tensor("out", [R, N], BF16, kind="ExternalOutput")

    with tile.TileContext(nc) as tc:
        with (
            tc.tile_pool(name="persist", bufs=1) as persist,
            tc.tile_pool(name="hTp", bufs=4) as hTp,
            tc.tile_pool(name="psB", bufs=4, space="PSUM") as psB,
            tc.tile_pool(name="psS", bufs=1, space="PSUM") as psS,
            tc.tile_pool(name="adjp", bufs=4) as adjp,
            tc.tile_pool(name="lp", bufs=2) as lp,
            tc.tile_pool(name="mnp", bufs=2) as mnp,
            tc.tile_pool(name="pp", bufs=2) as pp,
            tc.tile_pool(name="yp", bufs=2) as yp,
            tc.tile_pool(name="small", bufs=4) as small,
        ):
            # --------- setup: s1c (per-row bias) and bc2 (s2 broadcast) ------
            wT_sb = persist.tile([F_OUT, F_IN], F32)
            nc.scalar.dma_start(out=wT_sb, in_=wT_d[:, :])
            # a2[o, j] = a[j*64 + o]: a_src / a_dst as two columns
            a2 = persist.tile([F_OUT, 2], F32)
            a_t = a_d.tensor if hasattr(a_d, "tensor") else a_d
            nc.scalar.dma_start(
                out=a2, in_=bass.AP(tensor=a_t, offset=0, ap=[[1, F_OUT], [F_OUT, 2]])
            )
            hsT_sb = persist.tile([P, R], F16)
            nc.scalar.dma_start(out=hsT_sb, in_=hsT_d[:, :])
            hTs = []
            for g in range(8):
                hTc = hTp.tile([P, N // 8], F16, tag="hTc")
                nc.scalar.dma_start(
                    out=hTc, in_=hT_d[:, g * (N // 8):(g + 1) * (N // 8)]
                )
                hTs.append(hTc)

            # wa12[:, j] = w @ (a_src if j==0 else a_dst), one K=64 matmul
            ps_wa = psS.tile([P, 2], F32, tag="pswa")
            nc.tensor.matmul(ps_wa, lhsT=wT_sb, rhs=a2, start=True, stop=True)
            wa12 = persist.tile([P, 2], F32)
            nc.vector.tensor_copy(wa12, ps_wa)
            wa1h = persist.tile([P, 1], F16)
            nc.vector.tensor_copy(wa1h, wa12[:, 0:1])

            # w2b[k, p] = wa2[k] (fp16 stationary matrix for the bc2 matmuls)
            w2b = persist.tile([P, P], F16)
            nc.vector.memset(w2b, 1.0)
            nc.vector.tensor_scalar(
                out=w2b, in0=w2b, scalar1=wa12[:, 1:2], scalar2=None, op0=ALU.mult
            )

            negm = persist.tile([P, 1], F32)
            nc.vector.memset(negm, -MSHIFT)
            # warm the ACT Exp table during setup (off the critical path)
            warm = small.tile([P, 1], F32, tag="warm")
            nc.scalar.activation(out=warm, in_=negm, func=AF.Exp)

            # s1c[r, t] = s1[t*128 + r] (f32) for this core's 8 row tiles
            ps_s1 = psS.tile([P, RT], F32, tag="pss1")
            for t in range(RT):
                nc.tensor.matmul(
                    ps_s1[:, t:t + 1], lhsT=hsT_sb[:, t * P:(t + 1) * P],
                    rhs=wa1h, start=True, stop=True,
                )
            s1c = persist.tile([P, RT], F32)
            nc.vector.tensor_copy(s1c, ps_s1)

            # bc2[p, j] = s2[j] for all p, fp16 (16 chunks of 512 columns);
            # PSUM->SBUF downcast copies alternate DVE/ACT (idle during setup)
            bc2 = persist.tile([P, N], F16)
            for cg in range(16):
                psb = psB.tile([P, 512], F32, tag="psb")
                nc.tensor.matmul(
                    psb, lhsT=w2b,
                    rhs=hTs[cg // 2][:, (cg % 2) * 512:(cg % 2) * 512 + 512],
                    start=True, stop=True,
                )
                sl = slice(cg * 512, (cg + 1) * 512)
                if cg % 2 == 0:
                    nc.vector.tensor_copy(bc2[:, sl], psb)
                else:
                    nc.scalar.activation(
                        out=bc2[:, sl], in_=psb, func=AF.Copy
                    )

            # adj loads: SP HWDGE ring, full-width row tiles
            adjts = []
            for t in range(RT):
                adjt = adjp.tile([P, N], ADJ_DT, tag="adj")
                nc.sync.dma_start(out=adjt, in_=adj_d[t * P:(t + 1) * P, :])
                adjts.append(adjt)

            # ---------------- main loop over row tiles (sw-pipelined) --------
            # 3-stage pipeline so ACT's Prelu(t) fills the gap while DVE
            # finishes masking tile t-1:
            #   stage1(t):   DVE mn(t);  ACT Prelu(t) [0,FSP)
            #   stage2(t-1): DVE add/mul/max [FSP,N), tt-add mn;  ACT exp
            #   stage3(t-2): DVE S/recip/mult;  ACT-ring stores
            def stage1(t):
                mn = mnp.tile([P, N], F16, tag="mn")
                adjt = adjts[t]
                for hx in (0, 1):
                    sl = slice(hx * H, (hx + 1) * H)
                    nc.vector.tensor_scalar(
                        out=mn[:, sl], in0=adjt[:, sl], scalar1=0.0,
                        scalar2=MPEN, op0=ALU.is_le, op1=ALU.mult,
                    )
                l = lp.tile([P, N], F16, tag="l")
                nc.scalar.activation(
                    out=l[:, 0:FSP], in_=bc2[:, 0:FSP], func=AF.Prelu,
                    bias=s1c[:, t:t + 1], alpha=ALPHA,
                )
                return l, mn

            # stage2a: mask half 0 (issued at the END of iteration t so
            # ACT can open iteration t+1 with Exp h0 -- no DVE wait)
            def stage2a(t, l, mn):
                nc.vector.tensor_tensor(
                    out=l[:, 0:H], in0=l[:, 0:H], in1=mn[:, 0:H], op=ALU.add,
                )

            def stage2b(t, l):
                p = pp.tile([P, N], BF16, tag="p")
                S2 = small.tile([P, 2], F32, tag="S2")
                nc.scalar.activation(
                    out=p[:, 0:H], in_=l[:, 0:H], func=AF.Exp,
                    bias=negm[:, 0:1], accum_out=S2[:, 0:1],
                )
                return p, S2

            def stage2c(t, l, mn, p, S2):
                nc.vector.tensor_scalar(
                    out=l[:, FSP:N], in0=bc2[:, FSP:N],
                    scalar1=s1c[:, t:t + 1], scalar2=None, op0=ALU.add,
                )
                y = yp.tile([P, N - FSP], F16, tag="y")
                nc.vector.tensor_scalar(
                    out=y, in0=l[:, FSP:N], scalar1=ALPHA, scalar2=None,
                    op0=ALU.mult,
                )
                nc.vector.tensor_tensor(
                    out=l[:, FSP:N], in0=l[:, FSP:N], in1=y, op=ALU.max,
                )
                sl1 = slice(H, N)
                nc.vector.tensor_tensor(
                    out=l[:, sl1], in0=l[:, sl1], in1=mn[:, sl1], op=ALU.add,
                )
                nc.scalar.activation(
                    out=p[:, sl1], in_=l[:, sl1], func=AF.Exp,
                    bias=negm[:, 0:1], accum_out=S2[:, 1:2],
                )

            def stage3(t, p, S2):
                S = small.tile([P, 1], F32, tag="S")
                nc.vector.tensor_scalar(
                    out=S, in0=S2[:, 0:1], scalar1=S2[:, 1:2], scalar2=None,
                    op0=ALU.add,
                )
                rs = small.tile([P, 1], F32, tag="rs")
                nc.vector.reciprocal(rs, S)
                for hx in (0, 1):
                    sl = slice(hx * H, (hx + 1) * H)
                    nc.vector.tensor_scalar(
                        out=p[:, sl], in0=p[:, sl], scalar1=rs[:, 0:1],
                        scalar2=None, op0=ALU.mult,
                    )
                    nc.scalar.dma_start(
                        out=out_d[t * P:(t + 1) * P, sl], in_=p[:, sl]
                    )

            live = {}
            for t in range(RT + 2):
                if 1 <= t <= RT:
                    l, mn = live[t - 1]
                    p, S2 = stage2b(t - 1, l)
                    live[t - 1] = (l, mn, p, S2)
                if t < RT:
                    live[t] = stage1(t)
                if 1 <= t <= RT:
                    l, mn, p, S2 = live[t - 1]
                    stage2c(t - 1, l, mn, p, S2)
                    live[t - 1] = (p, S2)
                if t >= 2:
                    stage3(t - 2, *live.pop(t - 2))
                if t < RT:
                    stage2a(t, *live[t])

    nc.compile()
    return nc


def kernel(h, adj, w, a):
    global _CACHED_NC, LAST_RESULT
    h = np.ascontiguousarray(h, dtype=np.float32)
    adj = np.ascontiguousarray(adj, dtype=np.float32)
    w = np.ascontiguousarray(w, dtype=np.float32)
    a = np.ascontiguousarray(a, dtype=np.float32)

    if _CACHED_NC is None:
        _CACHED_NC = build_nc()
    nc = _CACHED_NC

    hT = np.ascontiguousarray(h.T.astype(np.float16))
    wT = np.ascontiguousarray(w.T)
    in_maps = [
        {
            "hT": hT,
            "hsT": np.ascontiguousarray(hT[:, i * R:(i + 1) * R]),
            "adj": np.ascontiguousarray(adj[i * R:(i + 1) * R].astype(ADJ_NP)),
            "wT": wT,
            "a": a,
        }
        for i in range(NCORES)
    ]
    res = run_bass_kernel_spmd(nc, in_maps, core_ids=list(range(NCORES)))
    LAST_RESULT = res
    return np.concatenate(
        [r["out"].astype(np.float32) for r in res.results], axis=0
    )
